# revision 20
# baseline (speedup 1.0000x reference)
"""Causal MHA (B=2, S=2048, D=2048, H=16) on 8 TRN2 NeuronCores.

Sharding: heads split across cores (2 heads/core, both batches). Each core:
  1. qk^T GEMM  : qT/kT in [feat(=head dim) partitions, token free] layout
  2. V GEMM     : V in natural [token partitions, feat free] layout
  3. causal attention (scoresT layout, ones-matmul softmax denominators,
     broadcast-matmul normalization; fully-masked tiles skipped)
  4. AllToAll   : heads -> token-slice redistribution of attention output
  5. out-proj   : full Wout on this core's 512-token slice -> outT slice

Matmuls run in float32r (TF32-like, ~1e-4 rel err, full PE rate for N>=256).
Host passes x^T, per-core W shards (attn scale folded into Wq), masks, ones.
Output is assembled on host from the 8 transposed token slices.
"""
import time

import numpy as np

import concourse.bacc as bacc
import concourse.mybir as mybir
import concourse.tile as tile
from concourse import bass_utils

# ---- problem constants (hardcoded; must match the reference) ----
B, S, D_MODEL, H = 2, 2048, 2048, 16
HEAD_DIM = 128
N_CORES = 8
CORE_IDS = list(range(N_CORES))
T = B * S                      # 4096 flattened tokens
HPC = H // N_CORES             # 2 heads per core
TOKB = 256                     # token block for phase-1 GEMM streaming
NTB = T // TOKB                # 16
NKC = D_MODEL // 128           # 16 contraction chunks of d_model
SQB = 512                      # sq block width in attention
NJ = S // SQB                  # 4 sq blocks per batch
NSK = S // 128                 # 16 sk chunks per batch
TSL = T // N_CORES             # 512-token output slice per core

F32 = mybir.dt.float32
F32R = mybir.dt.float32r
EXPF = mybir.ActivationFunctionType.Exp


def build(iters: int = 1, phases: str = "1234"):
    nc = bacc.Bacc("TRN2", target_bir_lowering=False, debug=False,
                   num_devices=N_CORES)

    xT_d = nc.dram_tensor("xT", [D_MODEL, T], F32R, kind="ExternalInput").ap()
    wqk_d = nc.dram_tensor("wqk", [D_MODEL, 4 * 128], F32R, kind="ExternalInput").ap()
    wv_d = nc.dram_tensor("wv", [D_MODEL, 2 * 128], F32R, kind="ExternalInput").ap()
    wout_d = nc.dram_tensor("wout", [NKC, NKC, 128, 128], F32R, kind="ExternalInput").ap()
    mask_d = nc.dram_tensor("masks", [4, 128, 2, SQB], F32R, kind="ExternalInput").ap()
    ones_d = nc.dram_tensor("ones", [128, 128], F32R, kind="ExternalInput").ap()
    outT_d = nc.dram_tensor("outT", [D_MODEL, TSL], F32, kind="ExternalOutput").ap()

    # internal DRAM for the all-to-all (f32r end to end; bypass moves bytes)
    a2a_in = nc.dram_tensor("a2a_in", [N_CORES, HPC * 128, TSL], F32R).ap()
    a2a_out = nc.dram_tensor("a2a_out", [N_CORES, HPC * 128, TSL], F32R).ap()

    with tile.TileContext(nc) as tc:
        with tc.tile_pool(name="persist", bufs=1) as pp:
            masks = pp.tile([128, 4, 2, SQB], F32R)
            ones = pp.tile([128, 128], F32R)
            nc.sync.dma_start(out=masks[:], in_=mask_d.rearrange("r p u q -> p r u q"))
            nc.sync.dma_start(out=ones[:], in_=ones_d[:])

            for _ in range(iters):
                _body(nc, tc, pp, xT_d, wqk_d, wv_d, wout_d, outT_d,
                      a2a_in, a2a_out, masks, ones, phases)

    nc.compile()
    return nc


def _body(nc, tc, pp, xT_d, wqk_d, wv_d, wout_d, outT_d, a2a_in, a2a_out,
          masks, ones, phases="1234"):
    noexp = "E" in phases
    noden = "D" in phases
    nomask = "M" in phases
    bigbufs = "B" in phases
    ebufs = 8 if bigbufs else 6
    plain = "P" in phases
    mmbufs = 4
    with tc.tile_pool(name="qkv", bufs=1) as qkvp, \
         tc.tile_pool(name="psum", bufs=1, space="PSUM") as psp:
        # persistent activations for this iteration
        qkT = qkvp.tile([128, 4, T], F32R)      # [d, (q0,q1,k0,k1), tok]
        v_sb = qkvp.tile([128, T // 128, 2 * 128], F32R)  # [tok%128, chunk, feat]

        # ---------------- phase 1: QKV projection ----------------
        with tc.tile_pool(name="p1", bufs=1) as p1:
            wqk = p1.tile([128, NKC, 4 * 128], F32R)
            wv = p1.tile([128, NKC, 2 * 128], F32R)
            nc.sync.dma_start(out=wqk[:], in_=wqk_d.rearrange("(k p) n -> p k n", p=128))
            nc.sync.dma_start(out=wv[:], in_=wv_d.rearrange("(k p) n -> p k n", p=128))
            xT_r = xT_d.rearrange("(k p) t -> p k t", p=128)

            for tb in range(NTB):
                xt = p1.tile([128, NKC, TOKB], F32R, tag="xt", bufs=2)
                nc.sync.dma_start(out=xt[:], in_=xT_r[:, :, tb * TOKB:(tb + 1) * TOKB])
                tok0 = tb * TOKB
                # q/k transposed GEMM: psum[feat, tok] += wqk_chunk.T @ xt_chunk
                for m in range(4):
                    ps = psp.tile([128, SQB], F32, tag="mm", bufs=mmbufs)
                    for kc in range(NKC):
                        nc.tensor.matmul(ps[:, :TOKB],
                                         wqk[:, kc, m * 128:(m + 1) * 128],
                                         xt[:, kc, :],
                                         start=(kc == 0), stop=(kc == NKC - 1))
                    nc.scalar.copy(qkT[:, m, tok0:tok0 + TOKB], ps[:, :TOKB])
                # V natural GEMM: psum[tok, feat] += xt_chunk.T(w) ... lhsT=xt
                for ti in range(TOKB // 128):
                    pv = psp.tile([128, SQB], F32, tag="acc", bufs=2)
                    for kc in range(NKC):
                        nc.tensor.matmul(pv[:, :256],
                                         xt[:, kc, ti * 128:(ti + 1) * 128],
                                         wv[:, kc, :],
                                         start=(kc == 0), stop=(kc == NKC - 1))
                    nc.vector.tensor_copy(v_sb[:, tb * 2 + ti, :], pv[:, :256])

        # ---------------- phase 2: causal attention ----------------
        if "2" not in phases:
            return
        with tc.tile_pool(name="p2", bufs=1) as p2:
            blocks = [(b, j) for j in range(NJ) for b in range(B)]
            sts = {}

            def emit_scores(b, j, c):
                for h in range(HPC):
                    st = psp.tile([128, SQB], F32, tag="mm", bufs=mmbufs,
                                  name=f"st{h}")
                    nc.tensor.matmul(
                        st[:],
                        qkT[:, 2 + h, b * S + c * 128: b * S + (c + 1) * 128],
                        qkT[:, h, b * S + j * SQB: b * S + (j + 1) * SQB],
                        start=True, stop=True)
                    sts[(c, h)] = st

            def emit_epilogue(b, j, o_accs, dens):
                for h in range(HPC):
                    o_sb = p2.tile([128, SQB], F32R, tag="osb", bufs=2,
                                   name=f"osb{h}")
                    if noden:
                        with nc.allow_low_precision(reason="timing variant only"):
                            nc.vector.tensor_copy(o_sb[:], o_accs[h][:])
                    else:
                        rec = p2.tile([1, SQB], F32R, tag="rec", bufs=2,
                                      name=f"rec{h}")
                        with nc.allow_low_precision(reason="softmax denom recip"):
                            nc.vector.reciprocal(rec[:], dens[h][:])
                        bc = psp.tile([128, SQB], F32, tag="mm", bufs=mmbufs,
                                      name=f"bc{h}")
                        nc.tensor.matmul(bc[:], ones[0:1, :], rec[:],
                                         start=True, stop=True)
                        bc_sb = p2.tile([128, SQB], F32, tag="bcsb", bufs=2,
                                        name=f"bcsb{h}")
                        nc.vector.tensor_copy(bc_sb[:], bc[:])
                        with nc.allow_low_precision(reason="f32r attn output"):
                            nc.vector.tensor_mul(o_sb[:], o_accs[h][:], bc_sb[:])
                    nc.sync.dma_start(
                        out=a2a_in[b * NJ + j, h * 128:(h + 1) * 128, :],
                        in_=o_sb[:])

            pending = None
            for (b, j) in blocks:
                cmax = 4 * j + 3
                o_accs, dens = {}, {}
                for h in range(HPC):
                    o_accs[h] = psp.tile([128, SQB], F32, tag="acc",
                                         bufs=2, name=f"oacc{h}")
                    dens[h] = None if noden else psp.tile(
                        [1, SQB], F32, tag="den", bufs=2, name=f"den{h}")
                emit_scores(b, j, 0)
                if pending is not None:
                    emit_epilogue(*pending)
                    pending = None
                for c in range(cmax + 1):
                    if c + 1 <= cmax:
                        emit_scores(b, j, c + 1)
                    for h in range(HPC):
                        st = sts.pop((c, h))
                        e = p2.tile([128, SQB], F32R, tag="exp", bufs=ebufs)
                        if noexp:
                            nc.scalar.copy(e[:], st[:])
                        else:
                            nc.scalar.activation(e[:], st[:], EXPF)
                        if c >= 4 * j and not nomask:
                            nc.vector.tensor_mul(e[:], e[:],
                                                 masks[:, c - 4 * j, 0, :])
                        nc.tensor.matmul(
                            o_accs[h][:],
                            v_sb[:, b * NSK + c, h * 128:(h + 1) * 128],
                            e[:], start=(c == 0), stop=(c == cmax))
                        if not noden:
                            nc.tensor.matmul(
                                dens[h][:], ones[:, 0:1], e[:],
                                start=(c == 0), stop=(c == cmax))
                pending = (b, j, o_accs, dens)
            emit_epilogue(*pending)

    # ---------------- phase 3: all-to-all ----------------
    if "3" in phases:
        nc.gpsimd.collective_compute(
            "AllToAll", mybir.AluOpType.bypass, replica_groups=[CORE_IDS],
            ins=[a2a_in[:]], outs=[a2a_out[:]])
    else:
        a2a_out = a2a_in

    # ---------------- phase 4: output projection ----------------
    if "4" not in phases:
        return
    with tc.tile_pool(name="p4", bufs=1) as p4, \
         tc.tile_pool(name="psum4", bufs=1, space="PSUM") as psp4:
        of = p4.tile([128, NKC, TSL], F32R)
        a2a_r = a2a_out.rearrange("g f t -> (g f) t").rearrange("(k p) t -> p k t", p=128)
        half = NKC // 2
        nc.sync.dma_start(out=of[:, :half, :], in_=a2a_r[:, :half, :])
        nc.gpsimd.dma_start(out=of[:, half:, :], in_=a2a_r[:, half:, :])
        for m in range(NKC):
            wm = p4.tile([128, NKC, 128], F32R, tag="wm", bufs=2)
            nc.sync.dma_start(out=wm[:], in_=wout_d[m].rearrange("k p n -> p k n"))
            wm_ap = wm[:]
            po = psp4.tile([128, TSL], F32, tag="po", bufs=2)
            for kc in range(NKC):
                nc.tensor.matmul(po[:], wm_ap[:, kc, :], of[:, kc, :],
                                 start=(kc == 0), stop=(kc == NKC - 1))
            ot = p4.tile([128, TSL], F32, tag="ot", bufs=2)
            nc.scalar.copy(ot[:], po[:])
            nc.sync.dma_start(out=outT_d[m * 128:(m + 1) * 128, :], in_=ot[:])


def _host_inputs(x, Wqkv, Wout):
    xT = np.ascontiguousarray(x.reshape(T, D_MODEL).T)
    scale = np.float32(HEAD_DIM ** -0.5)
    masks1 = np.zeros((4, 128, SQB), dtype=np.float32)
    for r in range(4):
        for i in range(128):
            lo = i + 128 * r
            if lo < SQB:
                masks1[r, i, lo:] = 1.0
    masks = np.ascontiguousarray(np.stack([masks1, masks1], axis=2))
    ones = np.ones((128, 128), dtype=np.float32)
    # [m, k, 128, 128]: tile (k,m) of Wout, so each m-chunk load is contiguous
    Wout_t = np.ascontiguousarray(
        Wout.astype(np.float32).reshape(NKC, 128, NKC, 128).transpose(2, 0, 1, 3))

    in_maps = []
    for c in range(N_CORES):
        cols_q = [Wqkv[:, (2 * c + h) * 128:(2 * c + h + 1) * 128] for h in range(HPC)]
        cols_k = [Wqkv[:, D_MODEL + (2 * c + h) * 128:D_MODEL + (2 * c + h + 1) * 128]
                  for h in range(HPC)]
        cols_v = [Wqkv[:, 2 * D_MODEL + (2 * c + h) * 128:2 * D_MODEL + (2 * c + h + 1) * 128]
                  for h in range(HPC)]
        wqk = np.concatenate([c_ * scale for c_ in cols_q] + cols_k, axis=1)
        wv = np.concatenate(cols_v, axis=1)
        in_maps.append({
            "xT": xT,
            "wqk": np.ascontiguousarray(wqk, dtype=np.float32),
            "wv": np.ascontiguousarray(wv, dtype=np.float32),
            "wout": Wout_t,
            "masks": masks,
            "ones": ones,
        })
    return in_maps


_NC_CACHE = {}


def _get_nc(iters=1, phases="1234"):
    key = (iters, phases)
    if key not in _NC_CACHE:
        _NC_CACHE[key] = build(iters, phases)
    return _NC_CACHE[key]


def kernel(x, Wqkv, Wout):
    x = np.asarray(x, dtype=np.float32)
    Wqkv = np.asarray(Wqkv, dtype=np.float32)
    Wout = np.asarray(Wout, dtype=np.float32)
    nc = _get_nc(1)
    in_maps = _host_inputs(x, Wqkv, Wout)
    res = None
    for attempt in range(3):
        try:
            res = bass_utils.run_bass_kernel_spmd(nc, in_maps, CORE_IDS)
            break
        except Exception:
            # transient NRT_EXEC_UNIT_UNRECOVERABLE after heavy prior device
            # activity recovers on retry; re-raise if persistent
            if attempt == 2:
                raise
            time.sleep(20)
    outT = np.concatenate([res.results[c]["outT"] for c in range(N_CORES)], axis=1)
    return np.ascontiguousarray(outT.T).reshape(B, S, D_MODEL)
